# revision 52
# baseline (speedup 1.0000x reference)
"""BotRGCN forward on 8 TRN2 NeuronCores (Bass/Tile SPMD kernel), v4.

Strategy (self-contained; shapes hardcoded for nn_BotRGCN1):
  - Nodes sharded 8-way (6272/core, N padded 50000->50176); f16 on-chip.
  - Dense MLPs node-parallel, feature-major on-chip ([128 feat, nodes]).
  - RGCN layer: aggregate-then-transform.  Edge messages gathered with
    dma_gather (f16 node rows, 256B; CHUNK=512 tokens/gather) and
    segment-summed on the TensorEngine via per-block one-hot matmuls.
    v4: the one-hot matrices are HOST-PRECOMPUTED narrow strips
    ([128 tok, span] f16, span = dst range of the sorted block, with the
    per-edge mean-normalization rcp FOLDED INTO the strip values).  They
    stream from DRAM once per layer; the DVE is_equal build and the
    per-group rcp drain multiply of v3 are gone.  Aggregation matmuls run
    narrow ([lo, lo+span) of the 384-wide dst tile).  Each (reg,tile,rel)
    group's PSUM is zeroed by a DVE memset first (narrow first matmul
    only clears has_written for the bank; content outside its N range
    must be real zeros for the transform read).
  - Transform = W_r^T @ acc on PE; all matmuls f16.
  - Boundary exchange: layer outputs split at local row 3072 into lo/hi
    halves; each half AllGathers as soon as it is ready so the collective
    overlaps compute.  Sources renumbered region-major (int16-safe).
  - Leaky ReLU = one scalar-engine Prelu (alpha=0.01) with fused bias.
  - Edge schedule = max over cores (SPMD: one program for all 8).
"""
import numpy as np
import ml_dtypes

N = 50000
M = 8
L = 6272            # nodes per core (N padded to 50176)
NPAD = M * L
D = 128
DDES = 768
R = 5
TW = 384            # dst tile width
NT = 17             # 16 full tiles + 1 of 128
RA = 3072           # region-a rows per core (tiles 0..7; MLP chunks 0..5)
RB = L - RA         # 3200 (tiles 8..16)
GA = M * RA         # rows in xf_a
GB = M * RB         # rows in xf_b
CHUNK = 1024        # tokens per dma_gather (2048 overflows the SWDGE ring)
MCH = 512           # MLP chunk width (13 chunks: 12x512 + 128)
NCH = 13
SLOPE = 0.01
NQUEUES = 4

_LAST = {}          # exec stats for test harness


def _tile_w(t):
    return min(TW, L - t * TW)


def _mch_w(c):
    return min(MCH, L - c * MCH)


def _prep_edges(edge_index, edge_type):
    """Per-core token streams + shared (max-over-cores) block schedule.

    Returns host-precomputed one-hot strips: for each 128-token block the
    [128, span] f16 matrix S with S[tok, dst - lo] = rcp(tok), used as the
    moving operand of the aggregation matmul."""
    src = np.asarray(edge_index[0], dtype=np.int64)
    dst = np.asarray(edge_index[1], dtype=np.int64)
    et = np.asarray(edge_type, dtype=np.int64)

    core = dst // L
    dloc = dst % L

    # group tokens per core: key = (region, tile, rel)
    per_core_groups = []
    for m in range(M):
        sel = np.nonzero(core == m)[0]
        s, dl, r = src[sel], dloc[sel], et[sel]
        cnt = np.zeros((R, L), np.float32)
        np.add.at(cnt, (r, dl), 1.0)
        rcp = 1.0 / np.maximum(cnt, 1.0)
        sc, sl = s // L, s % L
        reg = (sl >= RA).astype(np.int64)
        ridx = np.where(reg == 0, sc * RA + sl, sc * RB + (sl - RA))
        t = dl // TW
        key = (reg * NT + t) * R + r
        order = np.argsort(key, kind="stable")
        ridx, dl, r, key = ridx[order], dl[order], r[order], key[order]
        groups = {}
        bounds = np.searchsorted(key, np.arange(2 * NT * R + 1))
        for gk in range(2 * NT * R):
            a, b = bounds[gk], bounds[gk + 1]
            greg, gt, gr = gk // (NT * R), (gk // R) % NT, gk % R
            gidx = ridx[a:b].astype(np.int16)
            gdst = (dl[a:b] - gt * TW).astype(np.float32)
            grcp = rcp[r[a:b], dl[a:b]].astype(np.float32)
            # sort by dst within the group so each 128-token block spans a
            # narrow contiguous dst range (narrow strip + agg matmul)
            o2 = np.argsort(gdst, kind="stable")
            groups[(greg, gt, gr)] = (gidx[o2], gdst[o2], grcp[o2])
        per_core_groups.append(groups)

    # shared schedule: blocks per group = max over cores (>=1)
    nblk = {}
    for greg in range(2):
        for gt in range(NT):
            for gr in range(R):
                mx = max(len(per_core_groups[m][(greg, gt, gr)][0])
                         for m in range(M))
                nblk[(greg, gt, gr)] = max(1, -(-mx // 128))

    # segment order interleaves regions per tile half so tiles 0..7 finish by
    # ~50% of the layer (the y_lo AllGather for the next layer fires early)
    SEGS = [(0, 0, 8), (1, 0, 8), (0, 8, NT), (1, 8, NT)]

    # pad each segment to a CHUNK multiple by extending its last group with
    # pad blocks (gdst=1000 -> zero strip col -> harmless)
    for (sreg, tlo, thi) in SEGS:
        tot = 128 * sum(nblk[(sreg, gt, gr)]
                        for gt in range(tlo, thi) for gr in range(R))
        deficit = (-tot) % CHUNK
        nblk[(sreg, thi - 1, R - 1)] += deficit // 128

    # build padded per-core streams in segment-major order.
    # pad tokens use gdst=1000 (sorts last, strip value 0).
    TTOT = 128 * sum(nblk.values())
    NBLK = TTOT // 128
    gidx_all = np.zeros((M, TTOT), np.int16)
    gdst_all = np.full((M, TTOT), 1000.0, np.float32)
    grcp_all = np.zeros((M, TTOT), np.float32)
    pos = 0
    sched = []   # per block: (region, tile, rel, first, last, lo, span, soff)
    seg_tok = [0, 0, 0, 0]
    soff = 0
    for si, (sreg, tlo, thi) in enumerate(SEGS):
        for gt in range(tlo, thi):
            for gr in range(R):
                nb = nblk[(sreg, gt, gr)]
                for m in range(M):
                    gi, gd, gc = per_core_groups[m][(sreg, gt, gr)]
                    n = len(gi)
                    gidx_all[m, pos:pos + n] = gi
                    gdst_all[m, pos:pos + n] = gd
                    grcp_all[m, pos:pos + n] = gc
                for j in range(nb):
                    blk = gdst_all[:, pos + j * 128:pos + (j + 1) * 128]
                    real = blk[blk < 999.0]
                    if len(real):
                        lo, hi = int(real.min()), int(real.max()) + 1
                    else:
                        lo, hi = 0, 1
                    sched.append((sreg, gt, gr, j == 0, j == nb - 1,
                                  lo, hi - lo, soff))
                    soff += hi - lo
                pos += nb * 128
                seg_tok[si] += nb * 128
    assert pos == TTOT
    TOTCOL = soff

    # gather chunks: per segment, cut every CHUNK tokens
    chunks = []  # (region, tok_start, ntok)
    off = 0
    for si, (sreg, tlo, thi) in enumerate(SEGS):
        th = seg_tok[si]
        s0 = 0
        while s0 < th:
            n = min(CHUNK, th - s0)
            chunks.append((sreg, off + s0, n))
            s0 += n
        off += th

    # shift each block's dst values by its lo so every block compares against
    # the same iota window [0, MS) -> ONE batched DVE compare per chunk
    MS = max(s[6] for s in sched)
    for bi, (greg, gt, gr, first, last, lo, span, so) in enumerate(sched):
        t0 = bi * 128
        blk = gdst_all[:, t0:t0 + 128]
        pad = blk >= 999.0
        blk -= lo
        blk[pad] = 2000.0          # sentinel > MS: never matches iota
    gidx_w = np.tile(
        gidx_all.reshape(M, TTOT // 16, 16).transpose(0, 2, 1), (1, 8, 1)
    ).copy()                                            # [M, 128, TTOT//16]
    NBLK = TTOT // 128
    gdst_w = gdst_all.reshape(M, NBLK, 128).transpose(0, 2, 1).astype(np.float16).copy()
    grcp_w = grcp_all.reshape(M, NBLK, 128).transpose(0, 2, 1).astype(np.float16).copy()
    return gidx_w, gdst_w, grcp_w, sched, chunks, TTOT, NBLK, MS


def _build(sched, chunks, TTOT, NBLK, MS):
    from concourse import bacc, tile, mybir

    nc = bacc.Bacc("TRN2", target_bir_lowering=False, debug=False,
                   num_devices=M, num_swdge_queues=NQUEUES)
    f32, i16 = mybir.dt.float32, mybir.dt.int16
    f16 = mybir.dt.float16
    Alu = mybir.AluOpType
    Act = mybir.ActivationFunctionType

    desT_d = nc.dram_tensor("desT", [DDES, L], f16, kind="ExternalInput")
    gidx_d = nc.dram_tensor("gidx", [128, TTOT // 16], i16, kind="ExternalInput")
    gdst_d = nc.dram_tensor("gdst", [128, NBLK], f16, kind="ExternalInput")
    grcp_d = nc.dram_tensor("grcp", [128, NBLK], f16, kind="ExternalInput")
    wdes_d = nc.dram_tensor("wdes", [DDES, D], f16, kind="ExternalInput")
    win_d = nc.dram_tensor("win", [D, D], f16, kind="ExternalInput")
    wroot_d = nc.dram_tensor("wroot", [D, D], f16, kind="ExternalInput")
    wrel_d = nc.dram_tensor("wrel", [R, D, D], f16, kind="ExternalInput")
    wout1_d = nc.dram_tensor("wout1", [D, D], f16, kind="ExternalInput")
    wout2_d = nc.dram_tensor("wout2", [D, 2], f16, kind="ExternalInput")
    bias_d = nc.dram_tensor("bias", [D, 4], f32, kind="ExternalInput")  # des,in,rgcn,out1
    bout2_d = nc.dram_tensor("bout2", [2, 1], f32, kind="ExternalInput")
    out_d = nc.dram_tensor("out", [2, L], f32, kind="ExternalOutput")

    y_lo = [nc.dram_tensor(f"y{i}_lo", [RA, D], f16) for i in range(2)]
    y_hi = [nc.dram_tensor(f"y{i}_hi", [RB, D], f16) for i in range(2)]
    xf_a = [nc.dram_tensor(f"xf{i}_a", [GA, D], f16, addr_space="Shared")
            for i in range(2)]
    xf_b = [nc.dram_tensor(f"xf{i}_b", [GB, D], f16, addr_space="Shared")
            for i in range(2)]

    iota = nc.inline_tensor(
        np.broadcast_to(np.arange(MS, dtype=np.float16), (128, MS)).copy(), "iota")
    ident = nc.inline_tensor(np.eye(128, dtype=np.float16), "ident")

    with tile.TileContext(nc) as tc:
        with (
            tc.tile_pool(name="cst", bufs=1) as cst,
            tc.tile_pool(name="big", bufs=2) as big,
            tc.tile_pool(name="wk", bufs=4) as wk,
            tc.tile_pool(name="ps", bufs=1, space="PSUM") as psp,
        ):
            # ---- constants to SBUF ----
            iota_sb = cst.tile([128, MS], f16)
            nc.sync.dma_start(out=iota_sb[:], in_=iota[:])
            ident_sb = cst.tile([128, 128], f16)
            nc.sync.dma_start(out=ident_sb[:], in_=ident[:])
            gidx_sb = cst.tile([128, TTOT // 16], i16)
            nc.sync.dma_start(out=gidx_sb[:], in_=gidx_d[:])
            gdst_sb = cst.tile([128, NBLK], f16)
            nc.sync.dma_start(out=gdst_sb[:], in_=gdst_d[:])
            grcp_sb = cst.tile([128, NBLK], f16)
            nc.sync.dma_start(out=grcp_sb[:], in_=grcp_d[:])
            wdes_sb = cst.tile([128, 6, D], f16)
            for k in range(6):
                nc.sync.dma_start(out=wdes_sb[:, k, :], in_=wdes_d[k * 128:(k + 1) * 128, :])
            win_sb = cst.tile([128, D], f16)
            nc.sync.dma_start(out=win_sb[:], in_=win_d[:])
            wroot_sb = cst.tile([128, D], f16)
            nc.sync.dma_start(out=wroot_sb[:], in_=wroot_d[:])
            wrel_sb = cst.tile([128, R, D], f16)
            for r in range(R):
                nc.sync.dma_start(out=wrel_sb[:, r, :], in_=wrel_d[r])
            wout1_sb = cst.tile([128, D], f16)
            nc.sync.dma_start(out=wout1_sb[:], in_=wout1_d[:])
            wout2_sb = cst.tile([128, 2], f16)
            nc.sync.dma_start(out=wout2_sb[:], in_=wout2_d[:])
            bias_sb = cst.tile([128, 4], f32)
            nc.sync.dma_start(out=bias_sb[:], in_=bias_d[:])
            bout2_sb = cst.tile([2, 1], f32)
            nc.sync.dma_start(out=bout2_sb[:], in_=bout2_d[:])
            zeros_sb = cst.tile([128, TW], f16)
            nc.vector.memset(zeros_sb[:], 0.0)

            def all_gather(src_d, dst_d):
                nc.gpsimd.collective_compute(
                    "AllGather", mybir.AluOpType.bypass,
                    replica_groups=[list(range(M))],
                    ins=[src_d[:]], outs=[dst_d[:]])

            def transpose_store(src_f16_ap, row0, w, ylo_d, yhi_d):
                """feature-major f16 [128, w] -> node-major rows of y lo/hi."""
                for b in range(-(-w // 128)):
                    bw = min(128, w - b * 128)
                    trp = psp.tile([128, 128], f16, tag="tr")
                    nc.tensor.transpose(
                        trp[:bw, :], src_f16_ap[:, b * 128:b * 128 + bw], ident_sb[:])
                    ynm = wk.tile([128, D], f16, tag="ynm")
                    nc.scalar.activation(ynm[:bw, :], trp[:bw, :], Act.Copy)
                    r0 = row0 + b * 128
                    if r0 < RA:
                        nc.sync.dma_start(out=ylo_d[r0:r0 + bw, :], in_=ynm[:bw, :])
                    else:
                        nc.sync.dma_start(out=yhi_d[r0 - RA:r0 - RA + bw, :],
                                          in_=ynm[:bw, :])

            # ================= MLP =================
            x1T = big.tile([128, L], f16, tag="bigT")
            for c in range(NCH):
                w = _mch_w(c)
                ps = psp.tile([128, MCH], f32, tag="out", bufs=2)
                for k in range(6):
                    dt = wk.tile([128, MCH], f16, tag="des")
                    nc.sync.dma_start(
                        out=dt[:, :w],
                        in_=desT_d[k * 128:(k + 1) * 128, c * MCH:c * MCH + w])
                    nc.tensor.matmul(ps[:, :w], wdes_sb[:, k, :], dt[:, :w],
                                     start=(k == 0), stop=(k == 5))
                x0c = wk.tile([128, MCH], f16, tag="x0c")
                nc.scalar.activation(x0c[:, :w], ps[:, :w], Act.Prelu,
                                     bias=bias_sb[:, 0:1], alpha=SLOPE)
                ps2 = psp.tile([128, MCH], f32, tag="out", bufs=2)
                nc.tensor.matmul(ps2[:, :w], win_sb[:], x0c[:, :w],
                                 start=True, stop=True)
                nc.scalar.activation(x1T[:, c * MCH:c * MCH + w], ps2[:, :w],
                                     Act.Prelu, bias=bias_sb[:, 1:2], alpha=SLOPE)
                transpose_store(x1T[:, c * MCH:c * MCH + w], c * MCH, w,
                                y_lo[0], y_hi[0])
                if c == RA // MCH - 1:          # rows [0, RA) stored
                    all_gather(y_lo[0], xf_a[0])
            all_gather(y_hi[0], xf_b[0])

            # ================= RGCN layers =================
            qctr = [0]

            def rgcn_layer(xfa, xfb, x_curT, ylo_d, yhi_d, nxt):
                """nxt = (xf_a', xf_b') to AllGather into, or None if last."""
                yT = big.tile([128, L], f16, tag="bigT")
                xf_base = [xfa, xfb]
                agg = {}       # r -> psum tile for current (region, t)
                accTs = {}     # r -> drained SBUF acc for current (region, t)

                def finish_tile(reg, t):
                    w = _tile_w(t)
                    sl = yT[:, t * TW:t * TW + w]
                    ops = psp.tile([128, TW], f32, tag="out", bufs=2)
                    if reg == 0:
                        nc.tensor.matmul(ops[:, :w], wroot_sb[:],
                                         x_curT[:, t * TW:t * TW + w],
                                         start=True, stop=False)
                    else:
                        # pull region-a partial back into PSUM via identity
                        # matmul
                        nc.tensor.matmul(ops[:, :w], ident_sb[:], sl,
                                         start=True, stop=False)
                    for ri in range(R):
                        nc.tensor.matmul(ops[:, :w], wrel_sb[:, ri, :],
                                         accTs[ri][:, :w],
                                         start=False,
                                         stop=(ri == R - 1))
                    if reg == 0:
                        nc.scalar.activation(sl, ops[:, :w], Act.Identity,
                                             bias=bias_sb[:, 2:3])
                    else:
                        nc.scalar.activation(sl, ops[:, :w], Act.Copy)
                        if nxt is not None:
                            transpose_store(sl, t * TW, w, ylo_d, yhi_d)
                            if t == RA // TW - 1:
                                all_gather(ylo_d, nxt[0])
                            elif t == NT - 1:
                                all_gather(yhi_d, nxt[1])
                        else:
                            # last layer: fuse the output MLP per finished
                            # tile so it overlaps the remaining aggregation
                            ps_o = psp.tile([128, TW], f32, tag="out", bufs=2)
                            nc.tensor.matmul(ps_o[:, :w], wout1_sb[:], sl,
                                             start=True, stop=True)
                            z1 = wk.tile([128, TW], f16, tag="x0c")
                            nc.scalar.activation(z1[:, :w], ps_o[:, :w],
                                                 Act.Prelu,
                                                 bias=bias_sb[:, 3:4],
                                                 alpha=SLOPE)
                            ps2 = psp.tile([2, TW], f32, tag="out2")
                            nc.tensor.matmul(ps2[:, :w], wout2_sb[:],
                                             z1[:, :w], start=True, stop=True)
                            nc.scalar.activation(outT[:, t * TW:t * TW + w],
                                                 ps2[:, :w], Act.Identity,
                                                 bias=bout2_sb[:, 0:1])
                    accTs.clear()

                blk_i = 0
                cur = None  # (region, t)
                for (reg, s0, ntok) in chunks:
                    nb = ntok // 128
                    c0 = s0 // 128
                    g = wk.tile([128, CHUNK // 128, D], f16, tag="g", bufs=10)
                    q = qctr[0] % NQUEUES
                    nc.gpsimd.dma_gather(
                        out_ap=g[:, :nb, :],
                        in_ap=xf_base[reg][:],
                        idxs_ap=gidx_sb[:, s0 // 16:(s0 + ntok) // 16],
                        num_idxs=ntok,
                        num_idxs_reg=ntok,
                        elem_size=D,
                        queue_num=q,
                    )
                    qctr[0] += 1
                    # batched one-hot build for the whole chunk: dst values
                    # are pre-shifted by each block's lo so all nb blocks
                    # compare against iota[0:MS); 2 DVE ops per chunk
                    ind = wk.tile([128, CHUNK // 128, MS], f16, tag="ind",
                                  bufs=8)
                    i_bc = iota_sb[:, :MS].unsqueeze(1).broadcast_to(
                        (128, nb, MS))
                    g_bc = gdst_sb[:, c0:c0 + nb].unsqueeze(2).broadcast_to(
                        (128, nb, MS))
                    r_bc = grcp_sb[:, c0:c0 + nb].unsqueeze(2).broadcast_to(
                        (128, nb, MS))
                    nc.vector.tensor_tensor(out=ind[:, :nb, :], in0=i_bc,
                                            in1=g_bc, op=Alu.is_equal)
                    nc.vector.tensor_tensor(out=ind[:, :nb, :],
                                            in0=ind[:, :nb, :], in1=r_bc,
                                            op=Alu.mult)
                    for j in range(nb):
                        breg, bt, br, first, last, lo, span, so = sched[blk_i]
                        assert breg == reg
                        if cur is None:
                            cur = (breg, bt)
                        elif cur != (breg, bt):
                            finish_tile(*cur)
                            cur = (breg, bt)
                        w = _tile_w(bt)
                        if first:
                            agg[br] = psp.tile([128, TW], f32, tag="agg",
                                               name=f"agg{br}", bufs=4)
                            # matmuls run all-narrow with start=False; zero
                            # the content (via Scalar, keeping DVE free) so
                            # untouched columns read 0 and stale has_written
                            # bits accumulate onto zero
                            nc.scalar.activation(agg[br][:, :w],
                                                 zeros_sb[:, :w], Act.Copy)
                        nc.tensor.matmul(agg[br][:, lo:lo + span],
                                         g[:, j, :], ind[:, j, :span],
                                         start=False, stop=last)
                        if last:
                            acc = wk.tile([128, TW], f16, tag="accT", bufs=12)
                            nc.scalar.activation(acc[:, :w], agg[br][:, :w],
                                                 Act.Copy)
                            accTs[br] = acc
                        blk_i += 1
                finish_tile(*cur)
                assert blk_i == len(sched)
                return yT

            outT = big.tile([2, L], f32, tag="outT")
            y1T = rgcn_layer(xf_a[0], xf_b[0], x1T, y_lo[1], y_hi[1],
                             nxt=(xf_a[1], xf_b[1]))
            rgcn_layer(xf_a[1], xf_b[1], y1T, None, None, nxt=None)
            nc.sync.dma_start(out=out_d[:], in_=outT[:])

    nc.compile()
    return nc


def kernel(des, tweet, num_prop, cat_prop, edge_index, edge_type,
           W_des, b_des, W_in, b_in, W_rel, W_root, b_rgcn,
           W_out1, b_out1, W_out2, b_out2):
    import time
    from concourse.bass_utils import run_bass_kernel_spmd

    des = np.asarray(des, np.float32)
    gidx_w, gdst_w, grcp_w, sched, chunks, TTOT, NBLK, MS = _prep_edges(
        np.asarray(edge_index), np.asarray(edge_type))

    t0 = time.time()
    nc = _build(sched, chunks, TTOT, NBLK, MS)
    t1 = time.time()

    des_pad = np.zeros((NPAD, DDES), np.float16)
    des_pad[:N] = des.astype(np.float16)
    bias = np.stack([np.asarray(b_des, np.float32),
                     np.asarray(b_in, np.float32),
                     np.asarray(b_rgcn, np.float32),
                     np.asarray(b_out1, np.float32)], axis=1)  # [128,4]
    common = {
        "wdes": np.asarray(W_des, np.float16),
        "win": np.asarray(W_in, np.float16),
        "wroot": np.asarray(W_root, np.float16),
        "wrel": np.asarray(W_rel, np.float16),
        "wout1": np.asarray(W_out1, np.float16),
        "wout2": np.asarray(W_out2, np.float16),
        "bias": bias,
        "bout2": np.asarray(b_out2, np.float32).reshape(2, 1),
    }
    in_maps = []
    for m in range(M):
        in_maps.append({
            "desT": np.ascontiguousarray(des_pad[m * L:(m + 1) * L].T),
            "gidx": gidx_w[m], "gdst": gdst_w[m], "grcp": grcp_w[m],
            **common,
        })

    trace = bool(_LAST.get("trace"))
    res = run_bass_kernel_spmd(nc, in_maps, list(range(M)), trace=trace)
    t2 = time.time()
    _LAST["build_s"] = t1 - t0
    _LAST["run_s"] = t2 - t1
    _LAST["exec_ns"] = res.exec_time_ns
    _LAST["ttot"] = TTOT

    out = np.concatenate([res.results[m]["out"].T for m in range(M)], axis=0)
    return np.ascontiguousarray(out[:N])


# revision 55
# speedup vs baseline: 1.0690x; 1.0690x over previous
"""BotRGCN forward on 8 TRN2 NeuronCores (Bass/Tile SPMD kernel), v4.

Strategy (self-contained; shapes hardcoded for nn_BotRGCN1):
  - Nodes sharded 8-way (6272/core, N padded 50000->50176); f16 on-chip.
  - Dense MLPs node-parallel, feature-major on-chip ([128 feat, nodes]).
  - RGCN layer: aggregate-then-transform.  Edge messages gathered with
    dma_gather (f16 node rows, 256B; CHUNK=512 tokens/gather) and
    segment-summed on the TensorEngine via per-block one-hot matmuls.
    v4: the one-hot matrices are HOST-PRECOMPUTED narrow strips
    ([128 tok, span] f16, span = dst range of the sorted block, with the
    per-edge mean-normalization rcp FOLDED INTO the strip values).  They
    stream from DRAM once per layer; the DVE is_equal build and the
    per-group rcp drain multiply of v3 are gone.  Aggregation matmuls run
    narrow ([lo, lo+span) of the 384-wide dst tile).  Each (reg,tile,rel)
    group's PSUM is zeroed by a DVE memset first (narrow first matmul
    only clears has_written for the bank; content outside its N range
    must be real zeros for the transform read).
  - Transform = W_r^T @ acc on PE; all matmuls f16.
  - Boundary exchange: layer outputs split at local row 3072 into lo/hi
    halves; each half AllGathers as soon as it is ready so the collective
    overlaps compute.  Sources renumbered region-major (int16-safe).
  - Leaky ReLU = one scalar-engine Prelu (alpha=0.01) with fused bias.
  - Edge schedule = max over cores (SPMD: one program for all 8).
"""
import numpy as np
import ml_dtypes

N = 50000
M = 8
L = 6272            # nodes per core (N padded to 50176)
NPAD = M * L
D = 128
DDES = 768
R = 5
TW = 384            # dst tile width
NT = 17             # 16 full tiles + 1 of 128
RA = 3072           # region-a rows per core (tiles 0..7; MLP chunks 0..5)
RB = L - RA         # 3200 (tiles 8..16)
GA = M * RA         # rows in xf_a
GB = M * RB         # rows in xf_b
CHUNK = 1024        # tokens per dma_gather
SB = 2              # gather chunks per strip DMA (amortize DMA fixed cost)
MCH = 512           # MLP chunk width (13 chunks: 12x512 + 128)
NCH = 13
SLOPE = 0.01
NQUEUES = 4

_LAST = {}          # exec stats for test harness


def _tile_w(t):
    return min(TW, L - t * TW)


def _mch_w(c):
    return min(MCH, L - c * MCH)


def _prep_edges(edge_index, edge_type):
    """Per-core token streams + shared (max-over-cores) block schedule.

    Returns host-precomputed one-hot strips: for each 128-token block the
    [128, span] f16 matrix S with S[tok, dst - lo] = rcp(tok), used as the
    moving operand of the aggregation matmul."""
    src = np.asarray(edge_index[0], dtype=np.int64)
    dst = np.asarray(edge_index[1], dtype=np.int64)
    et = np.asarray(edge_type, dtype=np.int64)

    core = dst // L
    dloc = dst % L

    # group tokens per core: key = (region, tile, rel)
    per_core_groups = []
    for m in range(M):
        sel = np.nonzero(core == m)[0]
        s, dl, r = src[sel], dloc[sel], et[sel]
        cnt = np.zeros((R, L), np.float32)
        np.add.at(cnt, (r, dl), 1.0)
        rcp = 1.0 / np.maximum(cnt, 1.0)
        sc, sl = s // L, s % L
        reg = (sl >= RA).astype(np.int64)
        ridx = np.where(reg == 0, sc * RA + sl, sc * RB + (sl - RA))
        t = dl // TW
        key = (reg * NT + t) * R + r
        order = np.argsort(key, kind="stable")
        ridx, dl, r, key = ridx[order], dl[order], r[order], key[order]
        groups = {}
        bounds = np.searchsorted(key, np.arange(2 * NT * R + 1))
        for gk in range(2 * NT * R):
            a, b = bounds[gk], bounds[gk + 1]
            greg, gt, gr = gk // (NT * R), (gk // R) % NT, gk % R
            gidx = ridx[a:b].astype(np.int16)
            gdst = (dl[a:b] - gt * TW).astype(np.float32)
            grcp = rcp[r[a:b], dl[a:b]].astype(np.float32)
            # sort by dst within the group so each 128-token block spans a
            # narrow contiguous dst range (narrow strip + agg matmul)
            o2 = np.argsort(gdst, kind="stable")
            groups[(greg, gt, gr)] = (gidx[o2], gdst[o2], grcp[o2])
        per_core_groups.append(groups)

    # shared schedule: blocks per group = max over cores (>=1)
    nblk = {}
    for greg in range(2):
        for gt in range(NT):
            for gr in range(R):
                mx = max(len(per_core_groups[m][(greg, gt, gr)][0])
                         for m in range(M))
                nblk[(greg, gt, gr)] = max(1, -(-mx // 128))

    # pad each region to a CHUNK multiple by extending the last group with
    # pad blocks (gdst=1000 -> zero strip col -> harmless)
    for greg in range(2):
        tot = 128 * sum(nblk[(greg, gt, gr)]
                        for gt in range(NT) for gr in range(R))
        deficit = (-tot) % CHUNK
        nblk[(greg, NT - 1, R - 1)] += deficit // 128

    # build padded per-core streams in fixed (region, tile, rel) order.
    # pad tokens use gdst=1000 (sorts last, strip value 0).
    TTOT = 128 * sum(nblk.values())
    NBLK = TTOT // 128
    gidx_all = np.zeros((M, TTOT), np.int16)
    gdst_all = np.full((M, TTOT), 1000.0, np.float32)
    grcp_all = np.zeros((M, TTOT), np.float32)
    pos = 0
    sched = []   # per block: (region, tile, rel, first, last, lo, span, soff)
    reg_tok = [0, 0]
    soff = 0
    for greg in range(2):
        for gt in range(NT):
            for gr in range(R):
                nb = nblk[(greg, gt, gr)]
                for m in range(M):
                    gi, gd, gc = per_core_groups[m][(greg, gt, gr)]
                    n = len(gi)
                    gidx_all[m, pos:pos + n] = gi
                    gdst_all[m, pos:pos + n] = gd
                    grcp_all[m, pos:pos + n] = gc
                for j in range(nb):
                    blk = gdst_all[:, pos + j * 128:pos + (j + 1) * 128]
                    real = blk[blk < 999.0]
                    if len(real):
                        lo, hi = int(real.min()), int(real.max()) + 1
                    else:
                        lo, hi = 0, 1
                    sched.append((greg, gt, gr, j == 0, j == nb - 1,
                                  lo, hi - lo, soff))
                    soff += hi - lo
                pos += nb * 128
                reg_tok[greg] += nb * 128
    assert pos == TTOT
    TOTCOL = soff

    # gather chunks: per region, cut every CHUNK tokens
    chunks = []  # (region, tok_start, ntok)
    off = 0
    for greg in range(2):
        th = reg_tok[greg]
        s0 = 0
        while s0 < th:
            n = min(CHUNK, th - s0)
            chunks.append((greg, off + s0, n))
            s0 += n
        off += th

    # shift each block's dst values by its lo so every block compares against
    # the same iota window [0, MS) -> ONE batched DVE compare per chunk
    MS = max(s[6] for s in sched)
    for bi, (greg, gt, gr, first, last, lo, span, so) in enumerate(sched):
        t0 = bi * 128
        blk = gdst_all[:, t0:t0 + 128]
        pad = blk >= 999.0
        blk -= lo
        blk[pad] = 2000.0          # sentinel > MS: never matches iota
    gidx_w = np.tile(
        gidx_all.reshape(M, TTOT // 16, 16).transpose(0, 2, 1), (1, 8, 1)
    ).copy()                                            # [M, 128, TTOT//16]
    NBLK = TTOT // 128
    gdst_w = gdst_all.reshape(M, NBLK, 128).transpose(0, 2, 1).astype(np.float16).copy()
    grcp_w = grcp_all.reshape(M, NBLK, 128).transpose(0, 2, 1).astype(np.float16).copy()
    return gidx_w, gdst_w, grcp_w, sched, chunks, TTOT, NBLK, MS


def _build(sched, chunks, TTOT, NBLK, MS):
    from concourse import bacc, tile, mybir

    nc = bacc.Bacc("TRN2", target_bir_lowering=False, debug=False,
                   num_devices=M, num_swdge_queues=NQUEUES)
    f32, i16 = mybir.dt.float32, mybir.dt.int16
    f16 = mybir.dt.float16
    Alu = mybir.AluOpType
    Act = mybir.ActivationFunctionType

    desT_d = nc.dram_tensor("desT", [DDES, L], f16, kind="ExternalInput")
    gidx_d = nc.dram_tensor("gidx", [128, TTOT // 16], i16, kind="ExternalInput")
    gdst_d = nc.dram_tensor("gdst", [128, NBLK], f16, kind="ExternalInput")
    grcp_d = nc.dram_tensor("grcp", [128, NBLK], f16, kind="ExternalInput")
    wdes_d = nc.dram_tensor("wdes", [DDES, D], f16, kind="ExternalInput")
    win_d = nc.dram_tensor("win", [D, D], f16, kind="ExternalInput")
    wroot_d = nc.dram_tensor("wroot", [D, D], f16, kind="ExternalInput")
    wrel_d = nc.dram_tensor("wrel", [R, D, D], f16, kind="ExternalInput")
    wout1_d = nc.dram_tensor("wout1", [D, D], f16, kind="ExternalInput")
    wout2_d = nc.dram_tensor("wout2", [D, 2], f16, kind="ExternalInput")
    bias_d = nc.dram_tensor("bias", [D, 4], f32, kind="ExternalInput")  # des,in,rgcn,out1
    bout2_d = nc.dram_tensor("bout2", [2, 1], f32, kind="ExternalInput")
    out_d = nc.dram_tensor("out", [2, L], f32, kind="ExternalOutput")

    y_lo = [nc.dram_tensor(f"y{i}_lo", [RA, D], f16) for i in range(2)]
    y_hi = [nc.dram_tensor(f"y{i}_hi", [RB, D], f16) for i in range(2)]
    xf_a = [nc.dram_tensor(f"xf{i}_a", [GA, D], f16, addr_space="Shared")
            for i in range(2)]
    xf_b = [nc.dram_tensor(f"xf{i}_b", [GB, D], f16, addr_space="Shared")
            for i in range(2)]

    iota = nc.inline_tensor(
        np.broadcast_to(np.arange(MS, dtype=np.float16), (128, MS)).copy(), "iota")
    ident = nc.inline_tensor(np.eye(128, dtype=np.float16), "ident")

    with tile.TileContext(nc) as tc:
        with (
            tc.tile_pool(name="cst", bufs=1) as cst,
            tc.tile_pool(name="big", bufs=2) as big,
            tc.tile_pool(name="wk", bufs=4) as wk,
            tc.tile_pool(name="ps", bufs=1, space="PSUM") as psp,
        ):
            # ---- constants to SBUF ----
            iota_sb = cst.tile([128, MS], f16)
            nc.sync.dma_start(out=iota_sb[:], in_=iota[:])
            ident_sb = cst.tile([128, 128], f16)
            nc.sync.dma_start(out=ident_sb[:], in_=ident[:])
            gidx_sb = cst.tile([128, TTOT // 16], i16)
            nc.sync.dma_start(out=gidx_sb[:], in_=gidx_d[:])
            gdst_sb = cst.tile([128, NBLK], f16)
            nc.sync.dma_start(out=gdst_sb[:], in_=gdst_d[:])
            grcp_sb = cst.tile([128, NBLK], f16)
            nc.sync.dma_start(out=grcp_sb[:], in_=grcp_d[:])
            wdes_sb = cst.tile([128, 6, D], f16)
            for k in range(6):
                nc.sync.dma_start(out=wdes_sb[:, k, :], in_=wdes_d[k * 128:(k + 1) * 128, :])
            win_sb = cst.tile([128, D], f16)
            nc.sync.dma_start(out=win_sb[:], in_=win_d[:])
            wroot_sb = cst.tile([128, D], f16)
            nc.sync.dma_start(out=wroot_sb[:], in_=wroot_d[:])
            wrel_sb = cst.tile([128, R, D], f16)
            for r in range(R):
                nc.sync.dma_start(out=wrel_sb[:, r, :], in_=wrel_d[r])
            wout1_sb = cst.tile([128, D], f16)
            nc.sync.dma_start(out=wout1_sb[:], in_=wout1_d[:])
            wout2_sb = cst.tile([128, 2], f16)
            nc.sync.dma_start(out=wout2_sb[:], in_=wout2_d[:])
            bias_sb = cst.tile([128, 4], f32)
            nc.sync.dma_start(out=bias_sb[:], in_=bias_d[:])
            bout2_sb = cst.tile([2, 1], f32)
            nc.sync.dma_start(out=bout2_sb[:], in_=bout2_d[:])
            zeros_sb = cst.tile([128, TW], f16)
            nc.vector.memset(zeros_sb[:], 0.0)

            def all_gather(src_d, dst_d):
                nc.gpsimd.collective_compute(
                    "AllGather", mybir.AluOpType.bypass,
                    replica_groups=[list(range(M))],
                    ins=[src_d[:]], outs=[dst_d[:]])

            def transpose_store(src_f16_ap, row0, w, ylo_d, yhi_d):
                """feature-major f16 [128, w] -> node-major rows of y lo/hi."""
                for b in range(-(-w // 128)):
                    bw = min(128, w - b * 128)
                    trp = psp.tile([128, 128], f16, tag="tr")
                    nc.tensor.transpose(
                        trp[:bw, :], src_f16_ap[:, b * 128:b * 128 + bw], ident_sb[:])
                    ynm = wk.tile([128, D], f16, tag="ynm")
                    nc.scalar.activation(ynm[:bw, :], trp[:bw, :], Act.Copy)
                    r0 = row0 + b * 128
                    if r0 < RA:
                        nc.sync.dma_start(out=ylo_d[r0:r0 + bw, :], in_=ynm[:bw, :])
                    else:
                        nc.sync.dma_start(out=yhi_d[r0 - RA:r0 - RA + bw, :],
                                          in_=ynm[:bw, :])

            # ================= MLP =================
            x1T = big.tile([128, L], f16, tag="bigT")
            for c in range(NCH):
                w = _mch_w(c)
                ps = psp.tile([128, MCH], f32, tag="out", bufs=2)
                for k in range(6):
                    dt = wk.tile([128, MCH], f16, tag="des")
                    nc.sync.dma_start(
                        out=dt[:, :w],
                        in_=desT_d[k * 128:(k + 1) * 128, c * MCH:c * MCH + w])
                    nc.tensor.matmul(ps[:, :w], wdes_sb[:, k, :], dt[:, :w],
                                     start=(k == 0), stop=(k == 5))
                x0c = wk.tile([128, MCH], f16, tag="x0c")
                nc.scalar.activation(x0c[:, :w], ps[:, :w], Act.Prelu,
                                     bias=bias_sb[:, 0:1], alpha=SLOPE)
                ps2 = psp.tile([128, MCH], f32, tag="out", bufs=2)
                nc.tensor.matmul(ps2[:, :w], win_sb[:], x0c[:, :w],
                                 start=True, stop=True)
                nc.scalar.activation(x1T[:, c * MCH:c * MCH + w], ps2[:, :w],
                                     Act.Prelu, bias=bias_sb[:, 1:2], alpha=SLOPE)
                transpose_store(x1T[:, c * MCH:c * MCH + w], c * MCH, w,
                                y_lo[0], y_hi[0])
                if c == RA // MCH - 1:          # rows [0, RA) stored
                    all_gather(y_lo[0], xf_a[0])
            all_gather(y_hi[0], xf_b[0])

            # ================= RGCN layers =================
            qctr = [0]

            def rgcn_layer(xfa, xfb, x_curT, ylo_d, yhi_d, nxt):
                """nxt = (xf_a', xf_b') to AllGather into, or None if last."""
                yT = big.tile([128, L], f16, tag="bigT")
                xf_base = [xfa, xfb]
                agg = {}       # r -> psum tile for current (region, t)
                accTs = {}     # r -> drained SBUF acc for current (region, t)

                def finish_tile(reg, t):
                    w = _tile_w(t)
                    sl = yT[:, t * TW:t * TW + w]
                    ops = psp.tile([128, TW], f32, tag="out", bufs=2)
                    if reg == 0:
                        nc.tensor.matmul(ops[:, :w], wroot_sb[:],
                                         x_curT[:, t * TW:t * TW + w],
                                         start=True, stop=False)
                    else:
                        # pull region-a partial back into PSUM via identity
                        # matmul
                        nc.tensor.matmul(ops[:, :w], ident_sb[:], sl,
                                         start=True, stop=False)
                    for ri in range(R):
                        nc.tensor.matmul(ops[:, :w], wrel_sb[:, ri, :],
                                         accTs[ri][:, :w],
                                         start=False,
                                         stop=(ri == R - 1))
                    if reg == 0:
                        nc.scalar.activation(sl, ops[:, :w], Act.Identity,
                                             bias=bias_sb[:, 2:3])
                    else:
                        nc.scalar.activation(sl, ops[:, :w], Act.Copy)
                        if nxt is not None:
                            transpose_store(sl, t * TW, w, ylo_d, yhi_d)
                            if t == RA // TW - 1:
                                all_gather(ylo_d, nxt[0])
                            elif t == NT - 1:
                                all_gather(yhi_d, nxt[1])
                        else:
                            # last layer: fuse the output MLP per finished
                            # tile so it overlaps the remaining aggregation
                            ps_o = psp.tile([128, TW], f32, tag="out", bufs=2)
                            nc.tensor.matmul(ps_o[:, :w], wout1_sb[:], sl,
                                             start=True, stop=True)
                            z1 = wk.tile([128, TW], f16, tag="x0c")
                            nc.scalar.activation(z1[:, :w], ps_o[:, :w],
                                                 Act.Prelu,
                                                 bias=bias_sb[:, 3:4],
                                                 alpha=SLOPE)
                            ps2 = psp.tile([2, TW], f32, tag="out2")
                            nc.tensor.matmul(ps2[:, :w], wout2_sb[:],
                                             z1[:, :w], start=True, stop=True)
                            nc.scalar.activation(outT[:, t * TW:t * TW + w],
                                                 ps2[:, :w], Act.Identity,
                                                 bias=bout2_sb[:, 0:1])
                    accTs.clear()

                blk_i = 0
                cur = None  # (region, t)
                ind = None
                ind_off = 0
                for ci, (reg, s0, ntok) in enumerate(chunks):
                    nb = ntok // 128
                    c0 = s0 // 128
                    g = wk.tile([128, CHUNK // 128, D], f16, tag="g", bufs=10)
                    q = qctr[0] % NQUEUES
                    nc.gpsimd.dma_gather(
                        out_ap=g[:, :nb, :],
                        in_ap=xf_base[reg][:],
                        idxs_ap=gidx_sb[:, s0 // 16:(s0 + ntok) // 16],
                        num_idxs=ntok,
                        num_idxs_reg=ntok,
                        elem_size=D,
                        queue_num=q,
                    )
                    qctr[0] += 1
                    # batched one-hot build for a PAIR of chunks: dst values
                    # are pre-shifted by each block's lo so all blocks compare
                    # against iota[0:MS); 2 DVE ops per chunk pair
                    if ci % 2 == 0:
                        pnb = nb + (chunks[ci + 1][2] // 128
                                    if ci + 1 < len(chunks) else 0)
                        ind = wk.tile([128, 2 * CHUNK // 128, MS], f16,
                                      tag="ind", bufs=5)
                        i_bc = iota_sb[:, :MS].unsqueeze(1).broadcast_to(
                            (128, pnb, MS))
                        g_bc = gdst_sb[:, c0:c0 + pnb].unsqueeze(2).broadcast_to(
                            (128, pnb, MS))
                        r_bc = grcp_sb[:, c0:c0 + pnb].unsqueeze(2).broadcast_to(
                            (128, pnb, MS))
                        nc.vector.tensor_tensor(out=ind[:, :pnb, :], in0=i_bc,
                                                in1=g_bc, op=Alu.is_equal)
                        nc.vector.tensor_tensor(out=ind[:, :pnb, :],
                                                in0=ind[:, :pnb, :], in1=r_bc,
                                                op=Alu.mult)
                        ind_off = 0
                    else:
                        ind_off = chunks[ci - 1][2] // 128
                    for j in range(nb):
                        breg, bt, br, first, last, lo, span, so = sched[blk_i]
                        assert breg == reg
                        if cur is None:
                            cur = (breg, bt)
                        elif cur != (breg, bt):
                            finish_tile(*cur)
                            cur = (breg, bt)
                        w = _tile_w(bt)
                        if first:
                            agg[br] = psp.tile([128, TW], f32, tag="agg",
                                               name=f"agg{br}", bufs=4)
                            # matmuls run all-narrow with start=False; zero
                            # the content (via Scalar, keeping DVE free) so
                            # untouched columns read 0 and stale has_written
                            # bits accumulate onto zero
                            nc.scalar.activation(agg[br][:, :w],
                                                 zeros_sb[:, :w], Act.Copy)
                        nc.tensor.matmul(agg[br][:, lo:lo + span],
                                         g[:, j, :],
                                         ind[:, ind_off + j, :span],
                                         start=False, stop=last)
                        if last:
                            acc = wk.tile([128, TW], f16, tag="accT", bufs=12)
                            nc.scalar.activation(acc[:, :w], agg[br][:, :w],
                                                 Act.Copy)
                            accTs[br] = acc
                        blk_i += 1
                finish_tile(*cur)
                assert blk_i == len(sched)
                return yT

            outT = big.tile([2, L], f32, tag="outT")
            y1T = rgcn_layer(xf_a[0], xf_b[0], x1T, y_lo[1], y_hi[1],
                             nxt=(xf_a[1], xf_b[1]))
            rgcn_layer(xf_a[1], xf_b[1], y1T, None, None, nxt=None)
            nc.sync.dma_start(out=out_d[:], in_=outT[:])

    nc.compile()
    return nc


def kernel(des, tweet, num_prop, cat_prop, edge_index, edge_type,
           W_des, b_des, W_in, b_in, W_rel, W_root, b_rgcn,
           W_out1, b_out1, W_out2, b_out2):
    import time
    from concourse.bass_utils import run_bass_kernel_spmd

    des = np.asarray(des, np.float32)
    gidx_w, gdst_w, grcp_w, sched, chunks, TTOT, NBLK, MS = _prep_edges(
        np.asarray(edge_index), np.asarray(edge_type))

    t0 = time.time()
    nc = _build(sched, chunks, TTOT, NBLK, MS)
    t1 = time.time()

    des_pad = np.zeros((NPAD, DDES), np.float16)
    des_pad[:N] = des.astype(np.float16)
    bias = np.stack([np.asarray(b_des, np.float32),
                     np.asarray(b_in, np.float32),
                     np.asarray(b_rgcn, np.float32),
                     np.asarray(b_out1, np.float32)], axis=1)  # [128,4]
    common = {
        "wdes": np.asarray(W_des, np.float16),
        "win": np.asarray(W_in, np.float16),
        "wroot": np.asarray(W_root, np.float16),
        "wrel": np.asarray(W_rel, np.float16),
        "wout1": np.asarray(W_out1, np.float16),
        "wout2": np.asarray(W_out2, np.float16),
        "bias": bias,
        "bout2": np.asarray(b_out2, np.float32).reshape(2, 1),
    }
    in_maps = []
    for m in range(M):
        in_maps.append({
            "desT": np.ascontiguousarray(des_pad[m * L:(m + 1) * L].T),
            "gidx": gidx_w[m], "gdst": gdst_w[m], "grcp": grcp_w[m],
            **common,
        })

    trace = bool(_LAST.get("trace"))
    res = run_bass_kernel_spmd(nc, in_maps, list(range(M)), trace=trace)
    t2 = time.time()
    _LAST["build_s"] = t1 - t0
    _LAST["run_s"] = t2 - t1
    _LAST["exec_ns"] = res.exec_time_ns
    _LAST["ttot"] = TTOT

    out = np.concatenate([res.results[m]["out"].T for m in range(M)], axis=0)
    return np.ascontiguousarray(out[:N])


# revision 59
# speedup vs baseline: 1.0872x; 1.0171x over previous
"""BotRGCN forward on 8 TRN2 NeuronCores (Bass/Tile SPMD kernel), v4.

Strategy (self-contained; shapes hardcoded for nn_BotRGCN1):
  - Nodes sharded 8-way (6272/core, N padded 50000->50176); f16 on-chip.
  - Dense MLPs node-parallel, feature-major on-chip ([128 feat, nodes]).
  - RGCN layer: aggregate-then-transform.  Edge messages gathered with
    dma_gather (f16 node rows, 256B; CHUNK=512 tokens/gather) and
    segment-summed on the TensorEngine via per-block one-hot matmuls.
    v4: the one-hot matrices are HOST-PRECOMPUTED narrow strips
    ([128 tok, span] f16, span = dst range of the sorted block, with the
    per-edge mean-normalization rcp FOLDED INTO the strip values).  They
    stream from DRAM once per layer; the DVE is_equal build and the
    per-group rcp drain multiply of v3 are gone.  Aggregation matmuls run
    narrow ([lo, lo+span) of the 384-wide dst tile).  Each (reg,tile,rel)
    group's PSUM is zeroed by a DVE memset first (narrow first matmul
    only clears has_written for the bank; content outside its N range
    must be real zeros for the transform read).
  - Transform = W_r^T @ acc on PE; all matmuls f16.
  - Boundary exchange: layer outputs split at local row 3072 into lo/hi
    halves; each half AllGathers as soon as it is ready so the collective
    overlaps compute.  Sources renumbered region-major (int16-safe).
  - Leaky ReLU = one scalar-engine Prelu (alpha=0.01) with fused bias.
  - Edge schedule = max over cores (SPMD: one program for all 8).
"""
import numpy as np
import ml_dtypes

N = 50000
M = 8
L = 6272            # nodes per core (N padded to 50176)
NPAD = M * L
D = 128
DDES = 768
R = 5
TW = 384            # dst tile width
NT = 17             # 16 full tiles + 1 of 128
RA = 3072           # region-a rows per core (tiles 0..7; MLP chunks 0..5)
RB = L - RA         # 3200 (tiles 8..16)
GA = M * RA         # rows in xf_a
GB = M * RB         # rows in xf_b
CHUNK = 1024        # tokens per dma_gather
SB = 2              # gather chunks per strip DMA (amortize DMA fixed cost)
MCH = 512           # MLP chunk width (13 chunks: 12x512 + 128)
NCH = 13
SLOPE = 0.01
NQUEUES = 4

_LAST = {}          # exec stats for test harness


def _tile_w(t):
    return min(TW, L - t * TW)


def _mch_w(c):
    return min(MCH, L - c * MCH)


def _prep_edges(edge_index, edge_type):
    """Per-core token streams + shared (max-over-cores) block schedule.

    Returns host-precomputed one-hot strips: for each 128-token block the
    [128, span] f16 matrix S with S[tok, dst - lo] = rcp(tok), used as the
    moving operand of the aggregation matmul."""
    src = np.asarray(edge_index[0], dtype=np.int64)
    dst = np.asarray(edge_index[1], dtype=np.int64)
    et = np.asarray(edge_type, dtype=np.int64)

    core = dst // L
    dloc = dst % L

    # group tokens per core: key = (region, tile, rel)
    per_core_groups = []
    for m in range(M):
        sel = np.nonzero(core == m)[0]
        s, dl, r = src[sel], dloc[sel], et[sel]
        cnt = np.zeros((R, L), np.float32)
        np.add.at(cnt, (r, dl), 1.0)
        rcp = 1.0 / np.maximum(cnt, 1.0)
        sc, sl = s // L, s % L
        reg = (sl >= RA).astype(np.int64)
        ridx = np.where(reg == 0, sc * RA + sl, sc * RB + (sl - RA))
        t = dl // TW
        key = (reg * NT + t) * R + r
        order = np.argsort(key, kind="stable")
        ridx, dl, r, key = ridx[order], dl[order], r[order], key[order]
        groups = {}
        bounds = np.searchsorted(key, np.arange(2 * NT * R + 1))
        for gk in range(2 * NT * R):
            a, b = bounds[gk], bounds[gk + 1]
            greg, gt, gr = gk // (NT * R), (gk // R) % NT, gk % R
            gidx = ridx[a:b].astype(np.int16)
            gdst = (dl[a:b] - gt * TW).astype(np.float32)
            grcp = rcp[r[a:b], dl[a:b]].astype(np.float32)
            # sort by dst within the group so each 128-token block spans a
            # narrow contiguous dst range (narrow strip + agg matmul)
            o2 = np.argsort(gdst, kind="stable")
            groups[(greg, gt, gr)] = (gidx[o2], gdst[o2], grcp[o2])
        per_core_groups.append(groups)

    # shared schedule: blocks per group = max over cores (>=1)
    nblk = {}
    for greg in range(2):
        for gt in range(NT):
            for gr in range(R):
                mx = max(len(per_core_groups[m][(greg, gt, gr)][0])
                         for m in range(M))
                nblk[(greg, gt, gr)] = max(1, -(-mx // 128))

    # pad each region to a CHUNK multiple by extending the last group with
    # pad blocks (gdst=1000 -> zero strip col -> harmless)
    for greg in range(2):
        tot = 128 * sum(nblk[(greg, gt, gr)]
                        for gt in range(NT) for gr in range(R))
        deficit = (-tot) % CHUNK
        nblk[(greg, NT - 1, R - 1)] += deficit // 128

    # build padded per-core streams in fixed (region, tile, rel) order.
    # pad tokens use gdst=1000 (sorts last, strip value 0).
    TTOT = 128 * sum(nblk.values())
    NBLK = TTOT // 128
    gidx_all = np.zeros((M, TTOT), np.int16)
    gdst_all = np.full((M, TTOT), 1000.0, np.float32)
    grcp_all = np.zeros((M, TTOT), np.float32)
    pos = 0
    sched = []   # per block: (region, tile, rel, first, last, lo, span, soff)
    reg_tok = [0, 0]
    soff = 0
    for greg in range(2):
        for gt in range(NT):
            for gr in range(R):
                nb = nblk[(greg, gt, gr)]
                for m in range(M):
                    gi, gd, gc = per_core_groups[m][(greg, gt, gr)]
                    n = len(gi)
                    gidx_all[m, pos:pos + n] = gi
                    gdst_all[m, pos:pos + n] = gd
                    grcp_all[m, pos:pos + n] = gc
                w = _tile_w(gt)
                spans = []
                for j in range(nb):
                    blk = gdst_all[:, pos + j * 128:pos + (j + 1) * 128]
                    real = blk[blk < 999.0]
                    if len(real):
                        spans.append([int(real.min()), int(real.max()) + 1])
                    else:
                        spans.append(None)   # pad-only block
                # widen real blocks so the group's matmuls jointly write
                # EVERY column [0, w): the first starts at 0, each extends to
                # the next real block's lo, the last ends at w.  Untouched
                # columns then never exist, so no PSUM pre-zeroing is needed
                # (empty columns inside a matmul's N range get computed 0s).
                ridx_ = [j for j in range(nb) if spans[j] is not None]
                if ridx_:
                    spans[ridx_[0]][0] = 0
                    for a, b in zip(ridx_, ridx_[1:]):
                        spans[a][1] = max(spans[a][1], spans[b][0])
                    spans[ridx_[-1]][1] = w
                else:
                    spans[0] = [0, w]    # all-pad group: zero-ind full write
                for j in range(nb):
                    if spans[j] is None:
                        lo, hi = 0, 1    # pad block: rewrites col 0 with 0s
                    else:
                        lo, hi = spans[j]
                    sched.append((greg, gt, gr, j == 0, j == nb - 1,
                                  lo, hi - lo, soff))
                    soff += hi - lo
                pos += nb * 128
                reg_tok[greg] += nb * 128
    assert pos == TTOT
    TOTCOL = soff

    # gather chunks: per region, cut every CHUNK tokens
    chunks = []  # (region, tok_start, ntok)
    off = 0
    for greg in range(2):
        th = reg_tok[greg]
        s0 = 0
        while s0 < th:
            n = min(CHUNK, th - s0)
            chunks.append((greg, off + s0, n))
            s0 += n
        off += th

    # shift each block's dst values by its lo so every block compares against
    # the same iota window [0, MS) -> ONE batched DVE compare per chunk
    MS = max(s[6] for s in sched)
    for bi, (greg, gt, gr, first, last, lo, span, so) in enumerate(sched):
        t0 = bi * 128
        blk = gdst_all[:, t0:t0 + 128]
        pad = blk >= 999.0
        blk -= lo
        blk[pad] = 2000.0          # sentinel > MS: never matches iota
    gidx_w = np.tile(
        gidx_all.reshape(M, TTOT // 16, 16).transpose(0, 2, 1), (1, 8, 1)
    ).copy()                                            # [M, 128, TTOT//16]
    NBLK = TTOT // 128
    gdst_w = gdst_all.reshape(M, NBLK, 128).transpose(0, 2, 1).astype(np.float16).copy()
    grcp_w = grcp_all.reshape(M, NBLK, 128).transpose(0, 2, 1).astype(np.float16).copy()
    return gidx_w, gdst_w, grcp_w, sched, chunks, TTOT, NBLK, MS


def _build(sched, chunks, TTOT, NBLK, MS):
    from concourse import bacc, tile, mybir

    nc = bacc.Bacc("TRN2", target_bir_lowering=False, debug=False,
                   num_devices=M, num_swdge_queues=NQUEUES)
    f32, i16 = mybir.dt.float32, mybir.dt.int16
    f16 = mybir.dt.float16
    Alu = mybir.AluOpType
    Act = mybir.ActivationFunctionType

    desT_d = nc.dram_tensor("desT", [DDES, L], f16, kind="ExternalInput")
    gidx_d = nc.dram_tensor("gidx", [128, TTOT // 16], i16, kind="ExternalInput")
    gdst_d = nc.dram_tensor("gdst", [128, NBLK], f16, kind="ExternalInput")
    grcp_d = nc.dram_tensor("grcp", [128, NBLK], f16, kind="ExternalInput")
    wdes_d = nc.dram_tensor("wdes", [DDES, D], f16, kind="ExternalInput")
    win_d = nc.dram_tensor("win", [D, D], f16, kind="ExternalInput")
    wroot_d = nc.dram_tensor("wroot", [D, D], f16, kind="ExternalInput")
    wrel_d = nc.dram_tensor("wrel", [R, D, D], f16, kind="ExternalInput")
    wout1_d = nc.dram_tensor("wout1", [D, D], f16, kind="ExternalInput")
    wout2_d = nc.dram_tensor("wout2", [D, 2], f16, kind="ExternalInput")
    bias_d = nc.dram_tensor("bias", [D, 4], f32, kind="ExternalInput")  # des,in,rgcn,out1
    bout2_d = nc.dram_tensor("bout2", [2, 1], f32, kind="ExternalInput")
    out_d = nc.dram_tensor("out", [2, L], f32, kind="ExternalOutput")

    y_lo = [nc.dram_tensor(f"y{i}_lo", [RA, D], f16) for i in range(2)]
    y_hi = [nc.dram_tensor(f"y{i}_hi", [RB, D], f16) for i in range(2)]
    xf_a = [nc.dram_tensor(f"xf{i}_a", [GA, D], f16, addr_space="Shared")
            for i in range(2)]
    xf_b = [nc.dram_tensor(f"xf{i}_b", [GB, D], f16, addr_space="Shared")
            for i in range(2)]

    iota = nc.inline_tensor(
        np.broadcast_to(np.arange(MS, dtype=np.float16), (128, MS)).copy(), "iota")
    ident = nc.inline_tensor(np.eye(128, dtype=np.float16), "ident")

    with tile.TileContext(nc) as tc:
        with (
            tc.tile_pool(name="cst", bufs=1) as cst,
            tc.tile_pool(name="big", bufs=2) as big,
            tc.tile_pool(name="wk", bufs=4) as wk,
            tc.tile_pool(name="ps", bufs=1, space="PSUM") as psp,
        ):
            # ---- constants to SBUF ----
            iota_sb = cst.tile([128, MS], f16)
            nc.sync.dma_start(out=iota_sb[:], in_=iota[:])
            ident_sb = cst.tile([128, 128], f16)
            nc.sync.dma_start(out=ident_sb[:], in_=ident[:])
            gidx_sb = cst.tile([128, TTOT // 16], i16)
            nc.sync.dma_start(out=gidx_sb[:], in_=gidx_d[:])
            gdst_sb = cst.tile([128, NBLK], f16)
            nc.sync.dma_start(out=gdst_sb[:], in_=gdst_d[:])
            grcp_sb = cst.tile([128, NBLK], f16)
            nc.sync.dma_start(out=grcp_sb[:], in_=grcp_d[:])
            wdes_sb = cst.tile([128, 6, D], f16)
            for k in range(6):
                nc.sync.dma_start(out=wdes_sb[:, k, :], in_=wdes_d[k * 128:(k + 1) * 128, :])
            win_sb = cst.tile([128, D], f16)
            nc.sync.dma_start(out=win_sb[:], in_=win_d[:])
            wroot_sb = cst.tile([128, D], f16)
            nc.sync.dma_start(out=wroot_sb[:], in_=wroot_d[:])
            wrel_sb = cst.tile([128, R, D], f16)
            for r in range(R):
                nc.sync.dma_start(out=wrel_sb[:, r, :], in_=wrel_d[r])
            wout1_sb = cst.tile([128, D], f16)
            nc.sync.dma_start(out=wout1_sb[:], in_=wout1_d[:])
            wout2_sb = cst.tile([128, 2], f16)
            nc.sync.dma_start(out=wout2_sb[:], in_=wout2_d[:])
            bias_sb = cst.tile([128, 4], f32)
            nc.sync.dma_start(out=bias_sb[:], in_=bias_d[:])
            bout2_sb = cst.tile([2, 1], f32)
            nc.sync.dma_start(out=bout2_sb[:], in_=bout2_d[:])


            def all_gather(src_d, dst_d):
                nc.gpsimd.collective_compute(
                    "AllGather", mybir.AluOpType.bypass,
                    replica_groups=[list(range(M))],
                    ins=[src_d[:]], outs=[dst_d[:]])

            def transpose_store(src_f16_ap, row0, w, ylo_d, yhi_d):
                """feature-major f16 [128, w] -> node-major rows of y lo/hi."""
                for b in range(-(-w // 128)):
                    bw = min(128, w - b * 128)
                    trp = psp.tile([128, 128], f16, tag="tr")
                    nc.tensor.transpose(
                        trp[:bw, :], src_f16_ap[:, b * 128:b * 128 + bw], ident_sb[:])
                    ynm = wk.tile([128, D], f16, tag="ynm")
                    nc.scalar.activation(ynm[:bw, :], trp[:bw, :], Act.Copy)
                    r0 = row0 + b * 128
                    if r0 < RA:
                        nc.sync.dma_start(out=ylo_d[r0:r0 + bw, :], in_=ynm[:bw, :])
                    else:
                        nc.sync.dma_start(out=yhi_d[r0 - RA:r0 - RA + bw, :],
                                          in_=ynm[:bw, :])

            # ================= MLP =================
            x1T = big.tile([128, L], f16, tag="bigT")
            for c in range(NCH):
                w = _mch_w(c)
                ps = psp.tile([128, MCH], f32, tag="out", bufs=2)
                for k in range(6):
                    dt = wk.tile([128, MCH], f16, tag="des")
                    nc.sync.dma_start(
                        out=dt[:, :w],
                        in_=desT_d[k * 128:(k + 1) * 128, c * MCH:c * MCH + w])
                    nc.tensor.matmul(ps[:, :w], wdes_sb[:, k, :], dt[:, :w],
                                     start=(k == 0), stop=(k == 5))
                x0c = wk.tile([128, MCH], f16, tag="x0c")
                nc.scalar.activation(x0c[:, :w], ps[:, :w], Act.Prelu,
                                     bias=bias_sb[:, 0:1], alpha=SLOPE)
                ps2 = psp.tile([128, MCH], f32, tag="out", bufs=2)
                nc.tensor.matmul(ps2[:, :w], win_sb[:], x0c[:, :w],
                                 start=True, stop=True)
                nc.scalar.activation(x1T[:, c * MCH:c * MCH + w], ps2[:, :w],
                                     Act.Prelu, bias=bias_sb[:, 1:2], alpha=SLOPE)
                transpose_store(x1T[:, c * MCH:c * MCH + w], c * MCH, w,
                                y_lo[0], y_hi[0])
                if c == RA // MCH - 1:          # rows [0, RA) stored
                    all_gather(y_lo[0], xf_a[0])
            all_gather(y_hi[0], xf_b[0])

            # ================= RGCN layers =================
            qctr = [0]

            def rgcn_layer(xfa, xfb, x_curT, ylo_d, yhi_d, nxt):
                """nxt = (xf_a', xf_b') to AllGather into, or None if last."""
                yT = big.tile([128, L], f16, tag="bigT")
                xf_base = [xfa, xfb]
                agg = {}       # r -> psum tile for current (region, t)
                accTs = {}     # r -> drained SBUF acc for current (region, t)

                def finish_tile(reg, t):
                    w = _tile_w(t)
                    sl = yT[:, t * TW:t * TW + w]
                    ops = psp.tile([128, TW], f32, tag="out", bufs=2)
                    if reg == 0:
                        nc.tensor.matmul(ops[:, :w], wroot_sb[:],
                                         x_curT[:, t * TW:t * TW + w],
                                         start=True, stop=False)
                    else:
                        # pull region-a partial back into PSUM via identity
                        # matmul
                        nc.tensor.matmul(ops[:, :w], ident_sb[:], sl,
                                         start=True, stop=False)
                    for ri in range(R):
                        nc.tensor.matmul(ops[:, :w], wrel_sb[:, ri, :],
                                         accTs[ri][:, :w],
                                         start=False,
                                         stop=(ri == R - 1))
                    if reg == 0:
                        nc.scalar.activation(sl, ops[:, :w], Act.Identity,
                                             bias=bias_sb[:, 2:3])
                    else:
                        nc.scalar.activation(sl, ops[:, :w], Act.Copy)
                        if nxt is not None:
                            transpose_store(sl, t * TW, w, ylo_d, yhi_d)
                            if t == RA // TW - 1:
                                all_gather(ylo_d, nxt[0])
                            elif t == NT - 1:
                                all_gather(yhi_d, nxt[1])
                        else:
                            # last layer: fuse the output MLP per finished
                            # tile so it overlaps the remaining aggregation
                            ps_o = psp.tile([128, TW], f32, tag="out", bufs=2)
                            nc.tensor.matmul(ps_o[:, :w], wout1_sb[:], sl,
                                             start=True, stop=True)
                            z1 = wk.tile([128, TW], f16, tag="x0c")
                            nc.scalar.activation(z1[:, :w], ps_o[:, :w],
                                                 Act.Prelu,
                                                 bias=bias_sb[:, 3:4],
                                                 alpha=SLOPE)
                            ps2 = psp.tile([2, TW], f32, tag="out2")
                            nc.tensor.matmul(ps2[:, :w], wout2_sb[:],
                                             z1[:, :w], start=True, stop=True)
                            nc.scalar.activation(outT[:, t * TW:t * TW + w],
                                                 ps2[:, :w], Act.Identity,
                                                 bias=bout2_sb[:, 0:1])
                    accTs.clear()

                blk_i = 0
                cur = None  # (region, t)
                for (reg, s0, ntok) in chunks:
                    nb = ntok // 128
                    c0 = s0 // 128
                    g = wk.tile([128, CHUNK // 128, D], f16, tag="g", bufs=10)
                    q = qctr[0] % NQUEUES
                    nc.gpsimd.dma_gather(
                        out_ap=g[:, :nb, :],
                        in_ap=xf_base[reg][:],
                        idxs_ap=gidx_sb[:, s0 // 16:(s0 + ntok) // 16],
                        num_idxs=ntok,
                        num_idxs_reg=ntok,
                        elem_size=D,
                        queue_num=q,
                    )
                    qctr[0] += 1
                    # batched one-hot build for the whole chunk: dst values
                    # are pre-shifted by each block's lo so all nb blocks
                    # compare against iota[0:MS); 2 DVE ops per chunk
                    ind = wk.tile([128, CHUNK // 128, MS], f16, tag="ind",
                                  bufs=8)
                    i_bc = iota_sb[:, :MS].unsqueeze(1).broadcast_to(
                        (128, nb, MS))
                    g_bc = gdst_sb[:, c0:c0 + nb].unsqueeze(2).broadcast_to(
                        (128, nb, MS))
                    r_bc = grcp_sb[:, c0:c0 + nb].unsqueeze(2).broadcast_to(
                        (128, nb, MS))
                    nc.vector.tensor_tensor(out=ind[:, :nb, :], in0=i_bc,
                                            in1=g_bc, op=Alu.is_equal)
                    nc.vector.tensor_tensor(out=ind[:, :nb, :],
                                            in0=ind[:, :nb, :], in1=r_bc,
                                            op=Alu.mult)
                    for j in range(nb):
                        breg, bt, br, first, last, lo, span, so = sched[blk_i]
                        assert breg == reg
                        if cur is None:
                            cur = (breg, bt)
                        elif cur != (breg, bt):
                            finish_tile(*cur)
                            cur = (breg, bt)
                        w = _tile_w(bt)
                        if first:
                            agg[br] = psp.tile([128, TW], f32, tag="agg",
                                               name=f"agg{br}", bufs=4)
                        # group's widened spans jointly cover [0, w): the
                        # start=True first matmul clears the bank's
                        # has_written, later matmuls overwrite-or-accumulate,
                        # and every column gets a genuine (possibly zero)
                        # value -- no PSUM pre-zeroing needed
                        nc.tensor.matmul(agg[br][:, lo:lo + span],
                                         g[:, j, :], ind[:, j, :span],
                                         start=first, stop=last)
                        if last:
                            acc = wk.tile([128, TW], f16, tag="accT", bufs=12)
                            nc.scalar.activation(acc[:, :w], agg[br][:, :w],
                                                 Act.Copy)
                            accTs[br] = acc
                        blk_i += 1
                finish_tile(*cur)
                assert blk_i == len(sched)
                return yT

            outT = big.tile([2, L], f32, tag="outT")
            y1T = rgcn_layer(xf_a[0], xf_b[0], x1T, y_lo[1], y_hi[1],
                             nxt=(xf_a[1], xf_b[1]))
            rgcn_layer(xf_a[1], xf_b[1], y1T, None, None, nxt=None)
            nc.sync.dma_start(out=out_d[:], in_=outT[:])

    nc.compile()
    return nc


def kernel(des, tweet, num_prop, cat_prop, edge_index, edge_type,
           W_des, b_des, W_in, b_in, W_rel, W_root, b_rgcn,
           W_out1, b_out1, W_out2, b_out2):
    import time
    from concourse.bass_utils import run_bass_kernel_spmd

    des = np.asarray(des, np.float32)
    gidx_w, gdst_w, grcp_w, sched, chunks, TTOT, NBLK, MS = _prep_edges(
        np.asarray(edge_index), np.asarray(edge_type))

    t0 = time.time()
    nc = _build(sched, chunks, TTOT, NBLK, MS)
    t1 = time.time()

    des_pad = np.zeros((NPAD, DDES), np.float16)
    des_pad[:N] = des.astype(np.float16)
    bias = np.stack([np.asarray(b_des, np.float32),
                     np.asarray(b_in, np.float32),
                     np.asarray(b_rgcn, np.float32),
                     np.asarray(b_out1, np.float32)], axis=1)  # [128,4]
    common = {
        "wdes": np.asarray(W_des, np.float16),
        "win": np.asarray(W_in, np.float16),
        "wroot": np.asarray(W_root, np.float16),
        "wrel": np.asarray(W_rel, np.float16),
        "wout1": np.asarray(W_out1, np.float16),
        "wout2": np.asarray(W_out2, np.float16),
        "bias": bias,
        "bout2": np.asarray(b_out2, np.float32).reshape(2, 1),
    }
    in_maps = []
    for m in range(M):
        in_maps.append({
            "desT": np.ascontiguousarray(des_pad[m * L:(m + 1) * L].T),
            "gidx": gidx_w[m], "gdst": gdst_w[m], "grcp": grcp_w[m],
            **common,
        })

    trace = bool(_LAST.get("trace"))
    res = run_bass_kernel_spmd(nc, in_maps, list(range(M)), trace=trace)
    t2 = time.time()
    _LAST["build_s"] = t1 - t0
    _LAST["run_s"] = t2 - t1
    _LAST["exec_ns"] = res.exec_time_ns
    _LAST["ttot"] = TTOT

    out = np.concatenate([res.results[m]["out"].T for m in range(M)], axis=0)
    return np.ascontiguousarray(out[:N])
